# revision 2
# baseline (speedup 1.0000x reference)
"""Trainium2 Bass kernel for a feature-space attention head.

Reference computation (per batch b, with T=4096, E=1024, D=64):
    Q = x @ Wq; K = x @ Wk; V = x @ Wv            # (T,E)@(E,D) -> (T,D)
    R = (K^T @ Q) / sqrt(E)                        # (D,D) feature-space scores
    R = where(strictly_lower, -inf, R); R = softmax(R, axis=-1)
    out = V @ R                                    # (T,D)

Sharding: data-parallel over batch B=8 across the 8 NeuronCores (one batch
per core, no collectives).

Per-core device pipeline (bf16 operands, fp32 PSUM accumulation):
  - host pre-casts x and W to bf16; W is packed as w3 = [Wq/32 | Wk | Wv]
    (the 1/sqrt(E)=1/32 score scale folded into Wq), and the PE-transpose
    identity + softmax mask are uploaded as constants (no gpsimd setup).
  - x^T arrives directly via 32 XBAR transpose-DMAs (dma_start_transpose,
    DRAM->SBUF, one per (1024-row t-group, 128-wide e-chunk)) -- no PE
    transposes of x at all.
  - per 128-row t-tile: one joint QKV matmul pass (stationary = x^T chunk,
    moving = w3 [128,192]) -> QKV natural in PSUM; ACT copies to bf16;
    R += K^T Q accumulates in a persistent PSUM bank; V tiles re-transpose
    through the PE into a persistent V^T [64, T] buffer.
  - masked softmax on R (64x64) in fp32, O = V @ P via V^T-stationary
    chunks, per-group output DMA (fp32 out).
"""

import os
import sys

import numpy as np

for _p in ("/opt/trn_rl_repo", "/root/.axon_site/_ro/trn_rl_repo"):
    if os.path.isdir(_p) and _p not in sys.path:
        sys.path.append(_p)

import ml_dtypes  # noqa: E402

import concourse.bass as bass  # noqa: E402
import concourse.tile as tile  # noqa: E402
from concourse import bacc, mybir  # noqa: E402
from concourse.bass_utils import run_bass_kernel_spmd  # noqa: E402

B, T, E, D = 8, 4096, 1024, 64
N_CORES = 8
M3 = 3 * D                # 192: packed [Q|K|V] output columns
ECH = E // 128            # 8 e-chunks
NT = T // 128             # 32 t-tiles
GBLK = 1024               # t rows per XBAR transpose DMA
NG = T // GBLK            # 4 transpose-DMA groups
SPG = GBLK // 128         # 8 t-tiles per group

F32 = mybir.dt.float32
BF16 = mybir.dt.bfloat16
AX = mybir.AxisListType
AF = mybir.ActivationFunctionType

_COMPILED = None


def _build():
    nc = bacc.Bacc("TRN2", target_bir_lowering=False, debug=False,
                   num_devices=N_CORES)
    x = nc.dram_tensor("x", [T, E], BF16, kind="ExternalInput").ap()
    w3 = nc.dram_tensor("w3", [E, M3], BF16, kind="ExternalInput").ap()
    ident = nc.dram_tensor("ident", [128, 128], BF16,
                           kind="ExternalInput").ap()
    mask = nc.dram_tensor("mask", [64, 64], F32, kind="ExternalInput").ap()
    out = nc.dram_tensor("out", [T, D], F32, kind="ExternalOutput").ap()

    w3_r = w3.rearrange("(c p) m -> p c m", p=128)        # [128, 8, 192]
    out_r = out.rearrange("(c p) d -> p c d", p=128)      # [128, 32, 64]

    with tile.TileContext(nc) as tc:
        with (
            tc.tile_pool(name="const", bufs=1) as constp,
            tc.tile_pool(name="xt", bufs=NG * ECH) as xtp,
            tc.tile_pool(name="qkv", bufs=4) as qkvp,
            tc.tile_pool(name="vt", bufs=1) as vtp,
            tc.tile_pool(name="small", bufs=1) as smallp,
            tc.tile_pool(name="osb", bufs=2) as osbp,
            tc.tile_pool(name="ps_qkv", bufs=2, space="PSUM") as ps_qkv,
            tc.tile_pool(name="ps_vt", bufs=2, space="PSUM") as ps_vt,
            tc.tile_pool(name="ps_r", bufs=1, space="PSUM") as ps_rp,
            tc.tile_pool(name="ps_o", bufs=2, space="PSUM") as ps_o,
        ):
            # constants stream in while the x^T XBAR DMAs start: w3 on the
            # scalar HWDGE queue (earliest need), ident+mask via SWDGE
            w3_sb = constp.tile([128, ECH * M3], BF16)
            nc.scalar.dma_start(
                w3_sb[:].rearrange("p (c m) -> p c m", c=ECH), w3_r[:])
            ident_sb = constp.tile([128, 128], BF16)
            nc.gpsimd.dma_start(ident_sb[:], ident[:])
            mask_sb = constp.tile([64, 64], F32)
            nc.gpsimd.dma_start(mask_sb[:], mask[:])

            # x^T via hardware XBAR transpose: DRAM [GBLK, 128] -> SBUF
            # [128, GBLK]; all issued upfront on the sync queue, each lands
            # in its own contiguous tile (a strided destination would be
            # silently wrong on HW)
            xts = {}
            for g in range(NG):
                for j in range(ECH):
                    xt = xtp.tile([128, GBLK], BF16, tag="xt")
                    nc.sync.dma_start_transpose(
                        xt[:],
                        x[g * GBLK:(g + 1) * GBLK, j * 128:(j + 1) * 128],
                    )
                    xts[(g, j)] = xt

            w3v = w3_sb[:].rearrange("p (c m) -> p c m", c=ECH)
            vT = vtp.tile([64, T], BF16)          # persistent V^T
            ps_R = ps_rp.tile([64, 64], F32)      # persistent R accumulator

            pending = []  # [(qkv_sb, i)] R/Vt emission deferred one tile

            def emit_rv(qkv_sb, i):
                nc.tensor.matmul(
                    ps_R[:], qkv_sb[:, D:2 * D], qkv_sb[:, 0:D],
                    start=(i == 0), stop=(i == NT - 1),
                )
                pvt = ps_vt.tile([64, 128], BF16, tag="vt")
                nc.tensor.transpose(pvt[:], qkv_sb[:, 2 * D:3 * D],
                                    ident_sb[:])
                nc.vector.tensor_copy(vT[:, i * 128:(i + 1) * 128], pvt[:])

            for i in range(NT):
                g, s = i // SPG, i % SPG
                pq = ps_qkv.tile([128, M3], F32, tag="qkv")
                for j in range(ECH):
                    nc.tensor.matmul(
                        pq[:],
                        xts[(g, j)][:, s * 128:(s + 1) * 128],
                        w3v[:, j, :],
                        start=(j == 0), stop=(j == ECH - 1),
                    )
                qkv_sb = qkvp.tile([128, M3], BF16, tag="qkv_sb")
                nc.scalar.activation(qkv_sb[:], pq[:], AF.Copy)
                for args in pending:
                    emit_rv(*args)
                pending.clear()
                pending.append((qkv_sb, i))
            for args in pending:
                emit_rv(*args)
            pending.clear()

            # ---- softmax on R (64x64): fused mask-add from PSUM ----
            r_sb = smallp.tile([64, 64], F32)
            nc.vector.tensor_add(r_sb[:], ps_R[:], mask_sb[:])
            negmax = smallp.tile([64, 1], F32)
            nc.vector.reduce_max(negmax[:], r_sb[:], axis=AX.X, negate=True)
            p_exp = smallp.tile([64, 64], F32)
            rowsum = smallp.tile([64, 1], F32)
            nc.scalar.activation(p_exp[:], r_sb[:], AF.Exp,
                                 bias=negmax[:], scale=1.0, accum_out=rowsum[:])
            rinv = smallp.tile([64, 1], F32)
            nc.vector.reciprocal(rinv[:], rowsum[:])
            p_r = smallp.tile([64, 64], BF16)
            nc.vector.tensor_scalar_mul(p_r[:], p_exp[:], rinv[:])

            # ---- O = V @ P : lhsT = V^T chunks, rhs = P; DMA out per group ----
            for grp in range(4):
                po = ps_o.tile([128, 8 * D], F32, tag="o")
                for k in range(8):
                    c = grp * 8 + k
                    nc.tensor.matmul(
                        po[:, k * D:(k + 1) * D],
                        vT[:, c * 128:(c + 1) * 128], p_r[:],
                        start=True, stop=True,
                    )
                o_sb = osbp.tile([128, 8 * D], F32, tag="o_sb")
                if grp % 2 == 0:
                    nc.scalar.activation(o_sb[:], po[:], AF.Copy)
                else:
                    nc.vector.tensor_copy(o_sb[:], po[:])
                nc.sync.dma_start(
                    out_r[:, grp * 8:(grp + 1) * 8, :],
                    o_sb[:].rearrange("p (c d) -> p c d", c=8),
                )

    nc.compile()
    return nc


def _host_inputs(x, Wq, Wk, Wv):
    """Host-side prep: bf16 casts, weight packing, constant tables."""
    bf16 = ml_dtypes.bfloat16
    # fold the 1/sqrt(E) score scale into Wq (1/32 is exact in f32)
    w3 = np.ascontiguousarray(np.concatenate(
        [np.asarray(Wq, np.float32) * (1.0 / 32.0),
         np.asarray(Wk, np.float32),
         np.asarray(Wv, np.float32)], axis=1).astype(bf16))
    ident_h = np.eye(128, dtype=bf16)
    ii = np.arange(64)
    # additive mask: 0 where col >= row, -1e30 strictly below the diagonal
    mask_h = np.where(ii[None, :] >= ii[:, None], np.float32(0.0),
                      np.float32(-1e30)).astype(np.float32)
    xb = np.asarray(x, np.float32).astype(bf16)
    return [
        {"x": np.ascontiguousarray(xb[b]), "w3": w3, "ident": ident_h,
         "mask": mask_h}
        for b in range(B)
    ]


def kernel(x, Wq, Wk, Wv):
    global _COMPILED
    if _COMPILED is None:
        _COMPILED = _build()
    nc = _COMPILED

    in_maps = _host_inputs(x, Wq, Wk, Wv)
    res = run_bass_kernel_spmd(nc, in_maps, list(range(N_CORES)))
    return np.stack([res.results[b]["out"] for b in range(B)], axis=0)
